# revision 1
# baseline (speedup 1.0000x reference)
"""Single-head attention (B=8, N=2048, E=1024) on 8 TRN2 NeuronCores.

Sharding: data-parallel over batch — core i computes batch element i fully.
Host-side prep transposes x and W so the device kernel needs no transposes:
every matmul operand arrives with its contraction dim on SBUF partitions.

Per-core dataflow (all matmul compute in bf16, f32 PSUM accumulation):
  qT[f,n] = WT_lhsT.T @ xT_rhs      (projection, f on partitions)
  kT[f,n] = same
  v[n,e]  = xT_lhsT.T @ WTv_rhs     (natural layout, n on partitions)
  scoresT[j,i] = kT_lhsT.T @ qT_rhs ; expT = exp(scale*scoresT)  (ScalarE)
  denom[i] = ones-matmul over j-partitions of DVE-reduced exp sums
  out[i,e] = (expT_lhsT.T @ v_rhs) * (1/denom)
Softmax skips max-subtraction: scores are ~N(0,1) (max |s| < ~8), exp is
safe in f32 and softmax is shift-invariant.
"""

import numpy as np
import ml_dtypes

P = 128
E = 1024
N = 2048
F = 3072
KO = E // P      # 8 contraction subtiles
NT = N // P      # 16 row tiles
NCH = N // 512   # 4 chunks of 512
SCALE = 0.03125  # 1/sqrt(1024)

_CACHE = {}


def _build():
    import concourse.bacc as bacc
    import concourse.tile as tile
    import concourse.mybir as mybir

    f32 = mybir.dt.float32
    bf16 = mybir.dt.bfloat16
    AF = mybir.ActivationFunctionType
    Alu = mybir.AluOpType

    nc = bacc.Bacc("TRN2", target_bir_lowering=False, debug=False, num_devices=8)
    xT_d = nc.dram_tensor("xT", [E, N], bf16, kind="ExternalInput")
    WT_d = nc.dram_tensor("WT", [E, F], bf16, kind="ExternalInput")
    bqk_d = nc.dram_tensor("b_qk", [P, 16], f32, kind="ExternalInput")
    bv_d = nc.dram_tensor("b_v", [P, E], f32, kind="ExternalInput")
    out_d = nc.dram_tensor("out", [N, E], f32, kind="ExternalOutput")

    xT_r = xT_d.ap().rearrange("(ko p) (c n) -> c p ko n", p=P, n=512)
    WT_r = WT_d.ap().rearrange("(ko p) (s f) -> s p ko f", p=P, f=512)
    out_r = out_d.ap().rearrange("(it p) e -> it p e", p=P)

    with tile.TileContext(nc) as tc:
        with (
            tc.tile_pool(name="const", bufs=1) as const,
            tc.tile_pool(name="qkv", bufs=1) as qkv,
        ):
            bqk_t = const.tile([P, 16], f32, tag="bqk")
            nc.gpsimd.dma_start(bqk_t[:], bqk_d.ap())
            bv_t = const.tile([P, E], f32, tag="bv")
            nc.gpsimd.dma_start(bv_t[:], bv_d.ap())
            ones_t = const.tile([P, 1], bf16, tag="ones")
            nc.vector.memset(ones_t[:], 1.0)

            # qT/kT split per n-chunk so attention chunk ic only depends on
            # the chunks it reads (finer scheduling deps than one big tile)
            qTc = [
                qkv.tile([P, KO, 512], bf16, tag=f"qT{c}", name=f"qT{c}")
                for c in range(NCH)
            ]
            kTc = [
                qkv.tile([P, KO, 512], bf16, tag=f"kT{c}", name=f"kT{c}")
                for c in range(NCH)
            ]
            vt = qkv.tile([P, NT, E], bf16, tag="v")

            with (
                tc.tile_pool(name="pin", bufs=1) as pin,
                tc.tile_pool(name="pproj", bufs=7, space="PSUM") as pproj,
            ):
                # Section-sized input DMAs (one 1MB DMA per 512-col section,
                # [128, 8ko, 512] tiles), issued in consumption order: one
                # completion latency each instead of eight serialized ones.
                # xck[k][c]: x columns c*512..; wck[k][s]: W columns s*512..
                # (s 0-1: q, 2-3: k, 4-5: v).
                # Per-k chunked input DMAs in consumption order: the first
                # projection group streams k-slice by k-slice as DMA lands.
                # W section 0 rides the ACT HWDGE ring (separate physical
                # ring from SP) so it lands in parallel with x0.
                xck = [[None] * NCH for _ in range(KO)]
                wck = [[None] * 6 for _ in range(KO)]

                def load_x(c):
                    for k in range(KO):
                        t = pin.tile([P, 512], bf16, tag=f"x{k}_{c}", name=f"x{k}_{c}")
                        nc.sync.dma_start(t[:], xT_r[c][:, k, :])
                        xck[k][c] = t

                def load_w(s, eng):
                    for k in range(KO):
                        t = pin.tile([P, 512], bf16, tag=f"w{k}_{s}", name=f"w{k}_{s}")
                        eng.dma_start(t[:], WT_r[s][:, k, :])
                        wck[k][s] = t

                load_w(0, nc.scalar)
                load_x(0)
                load_w(1, nc.sync)
                load_w(2, nc.sync)
                load_w(3, nc.sync)
                load_x(1)
                load_w(4, nc.sync)
                load_w(5, nc.sync)
                load_x(2)
                load_x(3)

                def x_sl(c, k, fsl):
                    return xck[k][c][:, fsl]

                def w_sl(s, k, fsl):
                    return wck[k][s][:, fsl]

                # PE warmup: keep TensorE busy (and HAM at full clock) while
                # the first input chunks stream in. Results land in a junk
                # DRAM scratch so DCE keeps the chain.
                scratch = pin.tile([P, 512], bf16, tag="warm_in")
                nc.vector.memset(scratch[:], 0.0)
                junk_ps = pproj.tile([P, 512], f32, tag="warm_ps", bufs=1)
                for _ in range(28):
                    nc.tensor.matmul(
                        junk_ps[:], lhsT=scratch[:, :P], rhs=scratch[:],
                        start=True, stop=True,
                    )
                junk_sb = pin.tile([P, 1], f32, tag="warm_out")
                nc.vector.tensor_copy(junk_sb[:], junk_ps[:, 0:1])
                junk_d = nc.dram_tensor("warm_scratch", [P, 1], f32, kind="Internal")
                nc.sync.dma_start(junk_d.ap(), junk_sb[:])

                # q/k projection -> qT/kT [f(part), n], per-chunk tiles; the
                # last kT chunk's PSUM->SBUF copy drains during v-proj, so
                # attention never waits on it.
                for ch in range(NCH):
                    for ft in range(16):  # 0-7: q rows of W, 8-15: k rows
                        ps = pproj.tile([P, 512], f32, tag="ps")
                        for k in range(KO):
                            nc.tensor.matmul(
                                ps[:],
                                lhsT=w_sl(ft // 4, k, slice((ft % 4) * P, (ft % 4 + 1) * P)),
                                rhs=x_sl(ch, k, slice(0, 512)),
                                start=(k == 0),
                                stop=(k == KO - 1),
                            )
                        dst = (qTc if ft < 8 else kTc)[ch][:, ft % 8, :]
                        nc.scalar.activation(
                            dst, ps[:], AF.Identity, bias=bqk_t[:, ft : ft + 1], scale=1.0
                        )

                # v projection -> v [n(part), e]
                for nt in range(NT):
                    for ch2 in range(2):
                        esl = slice(ch2 * 512, (ch2 + 1) * 512)
                        ps = pproj.tile([P, 512], f32, tag="ps")
                        for k in range(KO):
                            nc.tensor.matmul(
                                ps[:],
                                lhsT=x_sl(nt // 4, k, slice((nt % 4) * P, (nt % 4 + 1) * P)),
                                rhs=w_sl(4 + ch2, k, slice(0, 512)),
                                start=(k == 0),
                                stop=(k == KO - 1),
                            )
                        nc.vector.tensor_tensor(
                            out=vt[:, nt, esl],
                            in0=ps[:],
                            in1=bv_t[:, esl],
                            op=Alu.add,
                        )

            with (
                tc.tile_pool(name="attn", bufs=2) as attn,
                tc.tile_pool(name="psc", bufs=2, space="PSUM") as psc,
                tc.tile_pool(name="pnum", bufs=4, space="PSUM") as pnum,
                tc.tile_pool(name="pden", bufs=2, space="PSUM") as pden,
            ):
                # Software pipeline: scores(ic) is emitted before the
                # denominator + numerator of (ic-1), so the DVE exp-sum
                # reduce of chunk ic-1 overlaps with scores matmuls of ic
                # instead of stalling PE.
                def emit_scores(ic):
                    expT = attn.tile([P, NT, 512], bf16, tag="expT", bufs=3)
                    for jt in range(NT):
                        ps = psc.tile([P, 512], f32, tag="ps_s")
                        for k in range(KO):
                            nc.tensor.matmul(
                                ps[:],
                                lhsT=kTc[jt // 4][:, k, (jt % 4) * P : (jt % 4 + 1) * P],
                                rhs=qTc[ic][:, k, :],
                                start=(k == 0),
                                stop=(k == KO - 1),
                            )
                        nc.scalar.activation(expT[:, jt, :], ps[:], AF.Exp, scale=SCALE)
                    # softmax denominators, step 1: sum over the 16 j-tiles
                    # (free-dim strided reduce on DVE)
                    sume = attn.tile([P, 512], f32, tag="sume")
                    nc.vector.reduce_sum(
                        sume[:],
                        expT.rearrange("p j i -> p i j"),
                        axis=mybir.AxisListType.X,
                    )
                    # bf16 copy so the cross-partition denominator matmul is a
                    # cheap bf16 op instead of a double-pass fp32 one. On DVE
                    # (not ACT): it waits on the reduce, and ACT's FIFO must
                    # stay clear for the next chunk's EXPs.
                    sume_bf = attn.tile([P, 512], bf16, tag="sume_bf")
                    nc.vector.tensor_copy(sume_bf[:], sume[:])
                    return expT, sume_bf

                def emit_tail(ic, expT, sume):
                    for isub in range(4):
                        it = ic * 4 + isub
                        # step 2: sum over the remaining 128 j-partitions
                        psd = pden.tile([P, 1], f32, tag="ps_d")
                        nc.tensor.matmul(
                            psd[:],
                            lhsT=sume[:, isub * P : (isub + 1) * P],
                            rhs=ones_t[:],
                            start=True,
                            stop=True,
                        )
                        rden = attn.tile([P, 1], f32, tag="rden", bufs=4)
                        nc.vector.reciprocal(rden[:], psd[:])
                        osb = attn.tile([P, E], f32, tag="osb", bufs=3)
                        for ch2 in range(2):
                            esl = slice(ch2 * 512, (ch2 + 1) * 512)
                            ps = pnum.tile([P, 512], f32, tag="ps_n")
                            for jt in range(NT):
                                nc.tensor.matmul(
                                    ps[:],
                                    lhsT=expT[:, jt, isub * P : (isub + 1) * P],
                                    rhs=vt[:, jt, esl],
                                    start=(jt == 0),
                                    stop=(jt == NT - 1),
                                )
                            # division on ScalarE (Copy with per-partition
                            # scale) keeps the DVE free so the pden PSUM slot
                            # recycles without stalling the next denom matmul
                            nc.scalar.activation(
                                osb[:, esl], ps[:], AF.Copy, scale=rden[:]
                            )
                            nc.sync.dma_start(out_r[it][:, esl], osb[:, esl])

                prev = None
                for ic in range(NCH):
                    cur = emit_scores(ic)
                    if prev is not None:
                        emit_tail(ic - 1, *prev)
                    prev = cur
                emit_tail(NCH - 1, *prev)
    nc.compile()
    return nc


def get_nc():
    if "nc" not in _CACHE:
        _CACHE["nc"] = _build()
    return _CACHE["nc"]


def prepare_in_maps(x, W_qkv, b_qkv):
    bf = ml_dtypes.bfloat16
    x = np.asarray(x, dtype=np.float32)
    W = np.asarray(W_qkv, dtype=np.float32)
    b = np.asarray(b_qkv, dtype=np.float32)
    assert x.shape == (8, N, E) and W.shape == (F, E) and b.shape == (F,)
    xT = np.ascontiguousarray(np.transpose(x, (0, 2, 1))).astype(bf)  # [8, E, N]
    WT = np.ascontiguousarray(W.T).astype(bf)  # [E, F]
    bqk = np.ascontiguousarray(b[: 2 * E].reshape(16, P).T)  # [P, 16]
    bv = np.ascontiguousarray(np.broadcast_to(b[2 * E :], (P, E)))  # [P, E]
    return [{"xT": xT[i], "WT": WT, "b_qk": bqk, "b_v": bv} for i in range(8)]


def kernel(x, W_qkv, b_qkv):
    from concourse.bass_utils import run_bass_kernel_spmd

    nc = get_nc()
    in_maps = prepare_in_maps(x, W_qkv, b_qkv)
    res = run_bass_kernel_spmd(nc, in_maps, core_ids=list(range(8)))
    return np.stack([res.results[i]["out"] for i in range(8)], axis=0)



# revision 2
# speedup vs baseline: 1.0345x; 1.0345x over previous
"""Single-head attention (B=8, N=2048, E=1024) on 8 TRN2 NeuronCores.

Sharding: data-parallel over batch - core i computes batch element i fully.

Weight-fusion restructuring: softmax(q k^T) depends on the weights only
through M = Wq^T Wk (plus a per-key bias correction), so M is folded on
the host (one E^3 sgemm of weight prep, shared by all batches/cores) and
the device computes
  uT[e2,i] = sum_e1 M[e1,e2] x[i,e1]          (256 matmuls)
  sT[j,i]  = sum_e2 x[j,e2] uT[e2,i]          (512)
instead of q-proj + k-proj + scores (512+512). Bias handling is exact:
  (q_i+bq).(k_j+bk) = x_i M x_j + [row-const terms that cancel in
  softmax] + c_j,  c_j = x_j.(Wk^T bq)
c_j is folded into the exp's per-partition bias on the host too.

Per-core dataflow (all matmul compute bf16, f32 PSUM):
  v    ->  vt[n(part), e] + bv         (DVE add drain)
  uT   ->  uTck[e2(part), i]           (ACT identity drain)
  expT[j(part), i] = exp(SCALE*sT + cb[j])   (ACT exp drain)
  sacc[j(part), i] += expT  per jt     (DVE running sum over j-tiles)
  denom: ones-matmul over j-partitions of sacc (after numerator it=0,
         so the DVE sum chain is off the PE critical path)
  out[i,e] = (expT.T @ v) * (1/denom)  (ACT copy-scale, DMA out)

Softmax skips max-subtraction: scores ~N(0,1), max |s| < ~15, exp fits
f32/bf16 range fine and softmax is shift-invariant.
"""

import numpy as np
import ml_dtypes

P = 128
E = 1024
N = 2048
KO = E // P      # 8 contraction subtiles
NT = N // P      # 16 row tiles
NCH = N // 512   # 4 chunks of 512
SCALE = 0.03125  # 1/sqrt(1024)
NWARM = 6

_CACHE = {}


def _build():
    import concourse.bacc as bacc
    import concourse.tile as tile
    import concourse.mybir as mybir

    f32 = mybir.dt.float32
    bf16 = mybir.dt.bfloat16
    AF = mybir.ActivationFunctionType
    Alu = mybir.AluOpType

    nc = bacc.Bacc("TRN2", target_bir_lowering=False, debug=False, num_devices=8)
    xT_d = nc.dram_tensor("xT", [E, N], bf16, kind="ExternalInput")
    m_d = nc.dram_tensor("m", [E, E], bf16, kind="ExternalInput")     # M[e1, e2]
    wv_d = nc.dram_tensor("wv", [E, E], bf16, kind="ExternalInput")   # [e_in, e_out] (= Wv^T)
    cb_d = nc.dram_tensor("cb", [P, 16], f32, kind="ExternalInput")
    bv_d = nc.dram_tensor("bv", [P, E], f32, kind="ExternalInput")
    out_d = nc.dram_tensor("out", [N, E], f32, kind="ExternalOutput")

    xT_r = xT_d.ap().rearrange("(ko p) (c n) -> c p ko n", p=P, n=512)
    m_r = m_d.ap().rearrange("(ko p) e -> ko p e", p=P)
    wv_r = wv_d.ap().rearrange("(ko p) e -> ko p e", p=P)
    out_r = out_d.ap().rearrange("(it p) e -> it p e", p=P)

    with tile.TileContext(nc) as tc:
        with (
            tc.tile_pool(name="const", bufs=1) as const,
            tc.tile_pool(name="big", bufs=1) as big,
        ):
            cb_t = const.tile([P, 16], f32, tag="cb")
            nc.gpsimd.dma_start(cb_t[:], cb_d.ap())
            bv_t = const.tile([P, E], f32, tag="bv")
            nc.gpsimd.dma_start(bv_t[:], bv_d.ap())
            ones_t = const.tile([P, 1], bf16, tag="ones")
            nc.vector.memset(ones_t[:], 1.0)

            # persistent SBUF tensors
            xck = [[None] * NCH for _ in range(KO)]  # x^T [e(part), n]
            for k in range(KO):
                for c in range(NCH):
                    xck[k][c] = big.tile([P, 512], bf16, tag=f"x{k}_{c}",
                                         name=f"x{k}_{c}")
            uTc = [big.tile([P, KO, 512], bf16, tag=f"uT{c}", name=f"uT{c}")
                   for c in range(NCH)]
            vt = big.tile([P, NT, E], bf16, tag="v")

            with (
                tc.tile_pool(name="pin", bufs=1) as pin,
                tc.tile_pool(name="pps1", bufs=8, space="PSUM") as pps1,
            ):
                wvt = [pin.tile([P, E], bf16, tag=f"wv{k}", name=f"wv{k}")
                       for k in range(KO)]
                mk = [pin.tile([P, E], bf16, tag=f"m{k}", name=f"m{k}")
                      for k in range(KO)]

                # DMA order = consumption order across 3 rings (~150GB/s
                # each while <=2 stream concurrently):
                #   ACT: xck c0, c1      (v-proj lhsT for nt 0..7)
                #   SP:  wv, xck c2, c3  (v-proj rhs first - it paces nt0)
                #   POOL: cb, bv, M      (M only needed at uT, ~35us slack)
                for k in range(KO):
                    nc.scalar.dma_start(xck[k][0][:], xT_r[0][:, k, :])
                for k in range(KO):
                    nc.sync.dma_start(wvt[k][:], wv_r[k])
                for k in range(KO):
                    nc.scalar.dma_start(xck[k][1][:], xT_r[1][:, k, :])
                for c in (2, 3):
                    for k in range(KO):
                        nc.sync.dma_start(xck[k][c][:], xT_r[c][:, k, :])
                for k in range(KO):
                    nc.gpsimd.dma_start(mk[k][:], m_r[k])

                # PE warmup: bridge the engine preamble until the first
                # (xck c0, wv) tiles land; keeps the clock ramp going.
                scratch = pin.tile([P, 512], bf16, tag="warm_in")
                nc.vector.memset(scratch[:], 0.0)
                junk_ps = None
                for _ in range(NWARM):
                    junk_ps = pps1.tile([P, 512], f32, tag="ps", name="ps_w")
                    nc.tensor.matmul(
                        junk_ps[:], lhsT=scratch[:, :P], rhs=scratch[:],
                        start=True, stop=True,
                    )
                junk_sb = pin.tile([P, 1], f32, tag="warm_out")
                nc.vector.tensor_copy(junk_sb[:], junk_ps[:, 0:1])
                junk_d = nc.dram_tensor("warm_scratch", [P, 1], f32, kind="Internal")
                nc.sync.dma_start(junk_d.ap(), junk_sb[:])

                # ---- v = x Wv^T + bv  [n(part), e]  (DVE drain) ----
                # First nt-block is k-OUTER (8 live banks): it consumes each
                # (xck c0, wv) k-slice pair right as the two DMA rings land
                # it, so v-proj streams behind the input DMA instead of
                # waiting ~14us for all of wv.
                psb = [pps1.tile([P, 512], f32, tag="ps", name=f"psv{_i}")
                       for _i in range(8)]
                for k in range(KO):
                    for nt in range(4):
                        for ech in range(2):
                            nc.tensor.matmul(
                                psb[nt * 2 + ech][:],
                                lhsT=xck[k][0][:, nt * P:(nt + 1) * P],
                                rhs=wvt[k][:, ech * 512:(ech + 1) * 512],
                                start=(k == 0),
                                stop=(k == KO - 1),
                            )
                for nt in range(4):
                    for ech in range(2):
                        esl = slice(ech * 512, (ech + 1) * 512)
                        nc.vector.tensor_tensor(
                            out=vt[:, nt, esl], in0=psb[nt * 2 + ech][:],
                            in1=bv_t[:, esl], op=Alu.add,
                        )
                for nt in range(4, NT):
                    ps = [pps1.tile([P, 512], f32, tag="ps", name=f"ps{_i}") for _i in range(2)]
                    for k in range(KO):
                        for ech in range(2):
                            nc.tensor.matmul(
                                ps[ech][:],
                                lhsT=xck[k][nt // 4][:, (nt % 4) * P:(nt % 4 + 1) * P],
                                rhs=wvt[k][:, ech * 512:(ech + 1) * 512],
                                start=(k == 0),
                                stop=(k == KO - 1),
                            )
                    for ech in range(2):
                        esl = slice(ech * 512, (ech + 1) * 512)
                        nc.vector.tensor_tensor(
                            out=vt[:, nt, esl], in0=ps[ech][:], in1=bv_t[:, esl],
                            op=Alu.add,
                        )

                # ---- uT = M^T x^T  [e2(part), i]  (ACT drain) ----
                for e2t in range(KO):
                    ps = [pps1.tile([P, 512], f32, tag="ps", name=f"ps{_i}") for _i in range(NCH)]
                    for k in range(KO):
                        for c in range(NCH):
                            nc.tensor.matmul(
                                ps[c][:],
                                lhsT=mk[k][:, e2t * P:(e2t + 1) * P],
                                rhs=xck[k][c][:],
                                start=(k == 0),
                                stop=(k == KO - 1),
                            )
                    for c in range(NCH):
                        nc.scalar.activation(
                            uTc[c][:, e2t, :], ps[c][:], AF.Identity, scale=1.0,
                        )

            with tc.tile_pool(name="attn", bufs=1) as attn:
                expT = [attn.tile([P, N], bf16, tag=f"expT{jt}", name=f"expT{jt}")
                        for jt in range(NT)]
                sacc = attn.tile([P, N], f32, tag="sacc")
                sume_bf = attn.tile([P, N], bf16, tag="sume_bf")
                rdent = attn.tile([P, 16], f32, tag="rdent")

                # ---- scoresT[j,i] = x M x^T, exp on ACT, running row-sums
                # over j-tiles on DVE ----
                with tc.tile_pool(name="psc", bufs=8, space="PSUM") as psc:
                    for jt in range(NT):
                        ps = [psc.tile([P, 512], f32, tag="ps_s", name=f"pss{_i}") for _i in range(NCH)]
                        for k in range(KO):
                            for c in range(NCH):
                                nc.tensor.matmul(
                                    ps[c][:],
                                    lhsT=xck[k][jt // 4][:, (jt % 4) * P:(jt % 4 + 1) * P],
                                    rhs=uTc[c][:, k, :],
                                    start=(k == 0),
                                    stop=(k == KO - 1),
                                )
                        for c in range(NCH):
                            nc.scalar.activation(
                                expT[jt][:, c * 512:(c + 1) * 512], ps[c][:],
                                AF.Exp, bias=cb_t[:, jt:jt + 1], scale=SCALE,
                            )
                        if jt == 0:
                            nc.vector.tensor_copy(sacc[:], expT[0][:])
                        else:
                            nc.vector.tensor_tensor(
                                out=sacc[:], in0=sacc[:], in1=expT[jt][:], op=Alu.add,
                            )

                # ---- numerator + scale + store ----
                # Denominator matmuls are emitted after it=0's numerator
                # group: they depend on the DVE sum chain (exp jt=15 ->
                # sacc -> sume_bf) which finishes ~5us after the last
                # scores matmul; emitting them first would stall the PE.
                with tc.tile_pool(name="pnum", bufs=4, space="PSUM") as pnum:
                    nc.vector.tensor_copy(sume_bf[:], sacc[:])
                    for it in range(NT):
                        ps = [pnum.tile([P, 512], f32, tag="ps_n", name=f"psn{_i}") for _i in range(2)]
                        for jt in range(NT):
                            for ech in range(2):
                                nc.tensor.matmul(
                                    ps[ech][:],
                                    lhsT=expT[jt][:, it * P:(it + 1) * P],
                                    rhs=vt[:, jt, ech * 512:(ech + 1) * 512],
                                    start=(jt == 0),
                                    stop=(jt == NT - 1),
                                )
                        if it == 0:
                            pd = pnum.tile([P, 16], f32, tag="pd", bufs=1)
                            for dt in range(NT):
                                nc.tensor.matmul(
                                    pd[:, dt:dt + 1],
                                    lhsT=sume_bf[:, dt * P:(dt + 1) * P],
                                    rhs=ones_t[:],
                                    start=True, stop=True,
                                )
                            nc.vector.reciprocal(rdent[:], pd[:])
                        osb = attn.tile([P, E], f32, tag="osb", bufs=3)
                        for ech in range(2):
                            esl = slice(ech * 512, (ech + 1) * 512)
                            nc.scalar.activation(
                                osb[:, esl], ps[ech][:], AF.Copy,
                                scale=rdent[:, it:it + 1],
                            )
                            nc.sync.dma_start(out_r[it][:, esl], osb[:, esl])
    nc.compile()
    return nc


def get_nc():
    if "nc" not in _CACHE:
        _CACHE["nc"] = _build()
    return _CACHE["nc"]


def prepare_in_maps(x, W_qkv, b_qkv):
    bf = ml_dtypes.bfloat16
    x = np.asarray(x, dtype=np.float32)
    W = np.asarray(W_qkv, dtype=np.float32)
    b = np.asarray(b_qkv, dtype=np.float32)
    assert x.shape == (8, N, E) and W.shape == (3 * E, E) and b.shape == (3 * E,)
    xT = np.ascontiguousarray(np.transpose(x, (0, 2, 1))).astype(bf)  # [8, E, N]
    # fused QK weight: scores depend on Wq, Wk only through M = Wq^T Wk
    m = np.ascontiguousarray(W[:E].T @ W[E:2 * E]).astype(bf)         # [e1, e2]
    wv = np.ascontiguousarray(W[2 * E:].T).astype(bf)                 # [e_in, e_out]
    bv = np.ascontiguousarray(np.broadcast_to(b[2 * E:], (P, E)))     # [P, E]
    # per-key score bias c_j = x_j . (Wk^T bq), folded into exp bias
    m1 = W[E:2 * E].T @ b[:E]                                         # [E]
    cb = SCALE * (x @ m1)                                             # [8, N]
    cb = np.ascontiguousarray(cb.reshape(8, 16, P).transpose(0, 2, 1)).astype(np.float32)
    return [{"xT": xT[i], "m": m, "wv": wv,
             "cb": cb[i], "bv": bv} for i in range(8)]


def kernel(x, W_qkv, b_qkv):
    from concourse.bass_utils import run_bass_kernel_spmd

    nc = get_nc()
    in_maps = prepare_in_maps(x, W_qkv, b_qkv)
    res = run_bass_kernel_spmd(nc, in_maps, core_ids=list(range(8)))
    return np.stack([res.results[i]["out"] for i in range(8)], axis=0)


# revision 3
# speedup vs baseline: 1.0357x; 1.0011x over previous
"""Single-head attention (B=8, N=2048, E=1024) on 8 TRN2 NeuronCores.

Sharding: data-parallel over batch - core i computes batch element i fully.

Weight-fusion restructuring: softmax(q k^T) depends on the weights only
through M = Wq^T Wk (plus a per-key bias correction), so M is folded on
the host (one E^3 sgemm of weight prep, shared by all batches/cores) and
the device computes
  uT[e2,i] = sum_e1 M[e1,e2] x[i,e1]          (256 matmuls)
  sT[j,i]  = sum_e2 x[j,e2] uT[e2,i]          (512)
instead of q-proj + k-proj + scores (512+512). Bias handling is exact:
  (q_i+bq).(k_j+bk) = x_i M x_j + [row-const terms that cancel in
  softmax] + c_j,  c_j = x_j.(Wk^T bq)
c_j is folded into the exp's per-partition bias on the host too.

Per-core dataflow (all matmul compute bf16, f32 PSUM):
  v    ->  vt[n(part), e] + bv         (DVE add drain)
  uT   ->  uTck[e2(part), i]           (ACT identity drain)
  expT[j(part), i] = exp(SCALE*sT + cb[j])   (ACT exp drain)
  sacc[j(part), i] += expT  per jt     (DVE running sum over j-tiles)
  denom: ones-matmul over j-partitions of sacc (after numerator it=0,
         so the DVE sum chain is off the PE critical path)
  out[i,e] = (expT.T @ v) * (1/denom)  (ACT copy-scale, DMA out)

Softmax skips max-subtraction: scores ~N(0,1), max |s| < ~15, exp fits
f32/bf16 range fine and softmax is shift-invariant.
"""

import numpy as np
import ml_dtypes

P = 128
E = 1024
N = 2048
KO = E // P      # 8 contraction subtiles
NT = N // P      # 16 row tiles
NCH = N // 512   # 4 chunks of 512
SCALE = 0.03125  # 1/sqrt(1024)
NWARM = 6

_CACHE = {}


def _build():
    import concourse.bacc as bacc
    import concourse.tile as tile
    import concourse.mybir as mybir

    f32 = mybir.dt.float32
    bf16 = mybir.dt.bfloat16
    AF = mybir.ActivationFunctionType
    Alu = mybir.AluOpType

    nc = bacc.Bacc("TRN2", target_bir_lowering=False, debug=False, num_devices=8)
    xT_d = nc.dram_tensor("xT", [E, N], bf16, kind="ExternalInput")
    m_d = nc.dram_tensor("m", [E, E], bf16, kind="ExternalInput")     # M[e1, e2]
    wv_d = nc.dram_tensor("wv", [E, E], bf16, kind="ExternalInput")   # [e_in, e_out] (= Wv^T)
    cb_d = nc.dram_tensor("cb", [P, 16], f32, kind="ExternalInput")
    bv_d = nc.dram_tensor("bv", [P, E], f32, kind="ExternalInput")
    out_d = nc.dram_tensor("out", [N, E], f32, kind="ExternalOutput")

    xT_r = xT_d.ap().rearrange("(ko p) (c n) -> c p ko n", p=P, n=512)
    m_r = m_d.ap().rearrange("(ko p) e -> ko p e", p=P)
    wv_r = wv_d.ap().rearrange("(ko p) e -> ko p e", p=P)
    out_r = out_d.ap().rearrange("(it p) e -> it p e", p=P)

    with tile.TileContext(nc) as tc:
        with (
            tc.tile_pool(name="const", bufs=1) as const,
            tc.tile_pool(name="big", bufs=1) as big,
        ):
            cb_t = const.tile([P, 16], f32, tag="cb")
            nc.gpsimd.dma_start(cb_t[:], cb_d.ap())
            bv_t = const.tile([P, E], f32, tag="bv")
            nc.gpsimd.dma_start(bv_t[:], bv_d.ap())
            ones_t = const.tile([P, 1], bf16, tag="ones")
            nc.vector.memset(ones_t[:], 1.0)

            # persistent SBUF tensors
            xck = [[None] * NCH for _ in range(KO)]  # x^T [e(part), n]
            for k in range(KO):
                for c in range(NCH):
                    xck[k][c] = big.tile([P, 512], bf16, tag=f"x{k}_{c}",
                                         name=f"x{k}_{c}")
            uTc = [big.tile([P, KO, 512], bf16, tag=f"uT{c}", name=f"uT{c}")
                   for c in range(NCH)]
            vt = big.tile([P, NT, E], bf16, tag="v")

            with (
                tc.tile_pool(name="pin", bufs=1) as pin,
                tc.tile_pool(name="pps1", bufs=8, space="PSUM") as pps1,
            ):
                wvt = [pin.tile([P, E], bf16, tag=f"wv{k}", name=f"wv{k}")
                       for k in range(KO)]
                mk = [pin.tile([P, E], bf16, tag=f"m{k}", name=f"m{k}")
                      for k in range(KO)]

                # DMA order = consumption order across 3 rings (~150GB/s
                # each while <=2 stream concurrently):
                #   ACT: xck c0, c1      (v-proj lhsT for nt 0..7)
                #   SP:  wv, xck c2, c3  (v-proj rhs first - it paces nt0)
                #   POOL: cb, bv, M      (M only needed at uT, ~35us slack)
                for k in range(KO):
                    nc.scalar.dma_start(xck[k][0][:], xT_r[0][:, k, :])
                for k in range(KO):
                    nc.sync.dma_start(wvt[k][:], wv_r[k])
                for k in range(KO):
                    nc.scalar.dma_start(xck[k][1][:], xT_r[1][:, k, :])
                for c in (2, 3):
                    for k in range(KO):
                        nc.sync.dma_start(xck[k][c][:], xT_r[c][:, k, :])
                # m rides the ACT ring behind xck c0/c1: it is only needed
                # at uT (~75us in), and streaming it on the POOL ring during
                # the v-proj head window would steal HBM bandwidth from the
                # (xck c0, wv) pair stream that paces the first nt-block.
                for k in range(KO):
                    nc.scalar.dma_start(mk[k][:], m_r[k])

                # PE warmup: bridge the engine preamble until the first
                # (xck c0, wv) tiles land; keeps the clock ramp going.
                scratch = pin.tile([P, 512], bf16, tag="warm_in")
                nc.vector.memset(scratch[:], 0.0)
                junk_ps = None
                for _ in range(NWARM):
                    junk_ps = pps1.tile([P, 512], f32, tag="ps", name="ps_w")
                    nc.tensor.matmul(
                        junk_ps[:], lhsT=scratch[:, :P], rhs=scratch[:],
                        start=True, stop=True,
                    )
                junk_sb = pin.tile([P, 1], f32, tag="warm_out")
                nc.vector.tensor_copy(junk_sb[:], junk_ps[:, 0:1])
                junk_d = nc.dram_tensor("warm_scratch", [P, 1], f32, kind="Internal")
                nc.sync.dma_start(junk_d.ap(), junk_sb[:])

                # ---- v = x Wv^T + bv  [n(part), e]  (DVE drain) ----
                # First nt-block is k-OUTER (8 live banks): it consumes each
                # (xck c0, wv) k-slice pair right as the two DMA rings land
                # it, so v-proj streams behind the input DMA instead of
                # waiting ~14us for all of wv.
                psb = [pps1.tile([P, 512], f32, tag="ps", name=f"psv{_i}")
                       for _i in range(8)]
                for k in range(KO):
                    for nt in range(4):
                        for ech in range(2):
                            nc.tensor.matmul(
                                psb[nt * 2 + ech][:],
                                lhsT=xck[k][0][:, nt * P:(nt + 1) * P],
                                rhs=wvt[k][:, ech * 512:(ech + 1) * 512],
                                start=(k == 0),
                                stop=(k == KO - 1),
                            )
                for nt in range(4):
                    for ech in range(2):
                        esl = slice(ech * 512, (ech + 1) * 512)
                        nc.vector.tensor_tensor(
                            out=vt[:, nt, esl], in0=psb[nt * 2 + ech][:],
                            in1=bv_t[:, esl], op=Alu.add,
                        )
                for nt in range(4, NT):
                    ps = [pps1.tile([P, 512], f32, tag="ps", name=f"ps{_i}") for _i in range(2)]
                    for k in range(KO):
                        for ech in range(2):
                            nc.tensor.matmul(
                                ps[ech][:],
                                lhsT=xck[k][nt // 4][:, (nt % 4) * P:(nt % 4 + 1) * P],
                                rhs=wvt[k][:, ech * 512:(ech + 1) * 512],
                                start=(k == 0),
                                stop=(k == KO - 1),
                            )
                    for ech in range(2):
                        esl = slice(ech * 512, (ech + 1) * 512)
                        nc.vector.tensor_tensor(
                            out=vt[:, nt, esl], in0=ps[ech][:], in1=bv_t[:, esl],
                            op=Alu.add,
                        )

                # ---- uT = M^T x^T  [e2(part), i]  (ACT drain) ----
                for e2t in range(KO):
                    ps = [pps1.tile([P, 512], f32, tag="ps", name=f"ps{_i}") for _i in range(NCH)]
                    for k in range(KO):
                        for c in range(NCH):
                            nc.tensor.matmul(
                                ps[c][:],
                                lhsT=mk[k][:, e2t * P:(e2t + 1) * P],
                                rhs=xck[k][c][:],
                                start=(k == 0),
                                stop=(k == KO - 1),
                            )
                    for c in range(NCH):
                        nc.scalar.activation(
                            uTc[c][:, e2t, :], ps[c][:], AF.Identity, scale=1.0,
                        )

            with tc.tile_pool(name="attn", bufs=1) as attn:
                expT = [attn.tile([P, N], bf16, tag=f"expT{jt}", name=f"expT{jt}")
                        for jt in range(NT)]
                sacc = attn.tile([P, N], f32, tag="sacc")
                sume_bf = attn.tile([P, N], bf16, tag="sume_bf")
                rdent = attn.tile([P, 16], f32, tag="rdent")

                # ---- scoresT[j,i] = x M x^T, exp on ACT, running row-sums
                # over j-tiles on DVE ----
                with tc.tile_pool(name="psc", bufs=8, space="PSUM") as psc:
                    for jt in range(NT):
                        ps = [psc.tile([P, 512], f32, tag="ps_s", name=f"pss{_i}") for _i in range(NCH)]
                        for k in range(KO):
                            for c in range(NCH):
                                nc.tensor.matmul(
                                    ps[c][:],
                                    lhsT=xck[k][jt // 4][:, (jt % 4) * P:(jt % 4 + 1) * P],
                                    rhs=uTc[c][:, k, :],
                                    start=(k == 0),
                                    stop=(k == KO - 1),
                                )
                        for c in range(NCH):
                            nc.scalar.activation(
                                expT[jt][:, c * 512:(c + 1) * 512], ps[c][:],
                                AF.Exp, bias=cb_t[:, jt:jt + 1], scale=SCALE,
                            )
                        if jt == 0:
                            nc.vector.tensor_copy(sacc[:], expT[0][:])
                        else:
                            nc.vector.tensor_tensor(
                                out=sacc[:], in0=sacc[:], in1=expT[jt][:], op=Alu.add,
                            )

                # ---- numerator + scale + store ----
                # Denominator matmuls are emitted after it=0's numerator
                # group: they depend on the DVE sum chain (exp jt=15 ->
                # sacc -> sume_bf) which finishes ~5us after the last
                # scores matmul; emitting them first would stall the PE.
                with tc.tile_pool(name="pnum", bufs=4, space="PSUM") as pnum:
                    nc.vector.tensor_copy(sume_bf[:], sacc[:])
                    for it in range(NT):
                        ps = [pnum.tile([P, 512], f32, tag="ps_n", name=f"psn{_i}") for _i in range(2)]
                        for jt in range(NT):
                            for ech in range(2):
                                nc.tensor.matmul(
                                    ps[ech][:],
                                    lhsT=expT[jt][:, it * P:(it + 1) * P],
                                    rhs=vt[:, jt, ech * 512:(ech + 1) * 512],
                                    start=(jt == 0),
                                    stop=(jt == NT - 1),
                                )
                        if it == 0:
                            pd = pnum.tile([P, 16], f32, tag="pd", bufs=1)
                            for dt in range(NT):
                                nc.tensor.matmul(
                                    pd[:, dt:dt + 1],
                                    lhsT=sume_bf[:, dt * P:(dt + 1) * P],
                                    rhs=ones_t[:],
                                    start=True, stop=True,
                                )
                            nc.vector.reciprocal(rdent[:], pd[:])
                        osb = attn.tile([P, E], f32, tag="osb", bufs=3)
                        for ech in range(2):
                            esl = slice(ech * 512, (ech + 1) * 512)
                            nc.scalar.activation(
                                osb[:, esl], ps[ech][:], AF.Copy,
                                scale=rdent[:, it:it + 1],
                            )
                            nc.sync.dma_start(out_r[it][:, esl], osb[:, esl])
    nc.compile()
    return nc


def get_nc():
    if "nc" not in _CACHE:
        _CACHE["nc"] = _build()
    return _CACHE["nc"]


def prepare_in_maps(x, W_qkv, b_qkv):
    bf = ml_dtypes.bfloat16
    x = np.asarray(x, dtype=np.float32)
    W = np.asarray(W_qkv, dtype=np.float32)
    b = np.asarray(b_qkv, dtype=np.float32)
    assert x.shape == (8, N, E) and W.shape == (3 * E, E) and b.shape == (3 * E,)
    xT = np.ascontiguousarray(np.transpose(x, (0, 2, 1))).astype(bf)  # [8, E, N]
    # fused QK weight: scores depend on Wq, Wk only through M = Wq^T Wk
    m = np.ascontiguousarray(W[:E].T @ W[E:2 * E]).astype(bf)         # [e1, e2]
    wv = np.ascontiguousarray(W[2 * E:].T).astype(bf)                 # [e_in, e_out]
    bv = np.ascontiguousarray(np.broadcast_to(b[2 * E:], (P, E)))     # [P, E]
    # per-key score bias c_j = x_j . (Wk^T bq), folded into exp bias
    m1 = W[E:2 * E].T @ b[:E]                                         # [E]
    cb = SCALE * (x @ m1)                                             # [8, N]
    cb = np.ascontiguousarray(cb.reshape(8, 16, P).transpose(0, 2, 1)).astype(np.float32)
    return [{"xT": xT[i], "m": m, "wv": wv,
             "cb": cb[i], "bv": bv} for i in range(8)]


def kernel(x, W_qkv, b_qkv):
    from concourse.bass_utils import run_bass_kernel_spmd

    nc = get_nc()
    in_maps = prepare_in_maps(x, W_qkv, b_qkv)
    res = run_bass_kernel_spmd(nc, in_maps, core_ids=list(range(8)))
    return np.stack([res.results[i]["out"] for i in range(8)], axis=0)
